# revision 4
# baseline (speedup 1.0000x reference)
"""Dense MLP kernel for Trainium2: y = inputs @ kernel + bias.

Full shapes: inputs (4, 2048, 4096) f32, kernel (4096, 16384) f32,
bias (16384,) f32 -> y (4, 2048, 16384) f32.

Strategy: tensor-parallel over the output feature dim F=16384, split 8
ways (2048 features per core). Each core receives the full activations
(pre-transposed on the host to [d, tok] tile layout and cast to bf16,
shared across all cores) plus its bf16 weight slice, computes
Y_c = X @ W_c + bias_c in fp32 PSUM, and the host concatenates the
per-core outputs along F. No device collectives.

Numerics: bf16 operands, fp32 PSUM accumulation and fp32 output:
measured 2.35e-3 L2 relative error at full scale (gate 2e-2).

Per-core program (single pass, W fully SBUF-resident):
- W slice held as 8 per-k-octile bf16 tiles [128k x 4ks x 2048f]
  (128KB/partition total), so each repeat iteration's weight reload
  can overlap the previous iteration's tail matmuls (per-tile WAR).
- Activations streamed once as 64 token tiles [128k x 32ks x 128t]
  bf16 (4-buffer pool, sync-engine DMA ring).
- Per token tile: 128 matmuls (32 k-subtiles x 4 f-chunks of 512);
  each stationary x tile feeds 4 N=512 matmuls into 4 PSUM banks
  (amortizes the bf16 embedded weight-load); PSUM pool rotates 2 sets
  so tt+1 matmuls overlap tt evictions.
- Bias added during PSUM->SBUF eviction on the vector engine; y stores
  issued on the scalar-engine DMA ring so store waits never block the
  x-load prefetch FIFO.

Measured 2.205 ms steady-state on 8 concurrent cores (repeat-loop
slope) vs 2.27 ms baseline. The 8-core sustained per-matmul rate is
power/clock-throttled (~265 ns vs ~225 ns single-core for the same
stream); structure choices beyond this point mostly trade energy, so
the kernel minimizes HBM traffic (bf16 in, 117MB/core total) and
wasted PE cycles (4-way stationary amortization).
"""

import numpy as np

B, S, D, F = 4, 2048, 4096, 16384
T = B * S
P = 128
NCORES = 8

FD = 512
FC = F // NCORES  # 2048
KS = D // P  # 32
NTT = T // P  # 64
NFC = FC // FD  # 4

_COMPILED = None


def _build(repeat=1):
    import concourse.bacc as bacc
    import concourse.mybir as mybir
    import concourse.tile as tile

    DT = mybir.dt.bfloat16
    nc = bacc.Bacc("TRN2", target_bir_lowering=False, debug=False)

    xt = nc.dram_tensor("xt", (P, NTT, KS, P), DT, kind="ExternalInput")
    w = nc.dram_tensor("w", (P, KS, FC), DT, kind="ExternalInput")
    bias = nc.dram_tensor("bias", (P, FC), mybir.dt.float32, kind="ExternalInput")
    y = nc.dram_tensor(
        "y", (P, NTT, NFC, FD), mybir.dt.float32, kind="ExternalOutput"
    )

    with tile.TileContext(nc) as tc:
        with (
            tc.tile_pool(name="wpool", bufs=1) as wpool,
            tc.tile_pool(name="bpool", bufs=1) as bpool,
            tc.tile_pool(name="xpool", bufs=4) as xpool,
            tc.tile_pool(name="opool", bufs=8) as opool,
            tc.tile_pool(name="pspool", bufs=2, space="PSUM") as pspool,
        ):
            def body():
                b_sb = bpool.tile([P, FC], mybir.dt.float32, name="b_sb")
                nc.scalar.dma_start(out=b_sb[:], in_=bias[:, :])
                gs = KS // 8
                w_sbs = []
                for g in range(8):
                    w_g = wpool.tile([P, gs, FC], DT, name=f"w_sb{g}")
                    nc.sync.dma_start(out=w_g[:], in_=w[:, g * gs : (g + 1) * gs, :])
                    w_sbs.append(w_g)
                for tt in range(NTT):
                    x_sb = xpool.tile([P, KS, P], DT, name="x_sb")
                    nc.sync.dma_start(out=x_sb[:], in_=xt[:, tt, :, :])
                    psums = [
                        pspool.tile([P, FD], mybir.dt.float32, name=f"ps{i}")
                        for i in range(NFC)
                    ]
                    for ks in range(KS):
                        for fc in range(NFC):
                            nc.tensor.matmul(
                                psums[fc][:],
                                lhsT=x_sb[:, ks, :],
                                rhs=w_sbs[ks // gs][:, ks % gs, fc * FD : (fc + 1) * FD],
                                start=(ks == 0),
                                stop=(ks == KS - 1),
                            )
                    for fc in range(NFC):
                        o_sb = opool.tile([P, FD], mybir.dt.float32, name="o_sb")
                        nc.vector.tensor_tensor(
                            out=o_sb[:],
                            in0=psums[fc][:],
                            in1=b_sb[:, fc * FD : (fc + 1) * FD],
                            op=mybir.AluOpType.add,
                        )
                        nc.scalar.dma_start(out=y[:, tt, fc, :], in_=o_sb[:])

            if repeat == 1:
                body()
            else:
                with tc.For_i(0, repeat, 1):
                    body()

    nc.compile()
    return nc


def _get_compiled():
    global _COMPILED
    if _COMPILED is None:
        _COMPILED = _build()
    return _COMPILED


def prep_inputs(inputs, kernel, bias):
    import ml_dtypes

    x32 = np.ascontiguousarray(np.asarray(inputs, dtype=np.float32).reshape(T, D))
    xt_host = np.ascontiguousarray(
        x32.reshape(NTT, P, KS, P).transpose(3, 0, 2, 1).astype(ml_dtypes.bfloat16)
    )
    w32 = np.asarray(kernel, dtype=np.float32)
    w_host = np.ascontiguousarray(
        w32.reshape(KS, P, F).transpose(1, 0, 2).astype(ml_dtypes.bfloat16)
    )
    b32 = np.asarray(bias, dtype=np.float32)
    in_maps = []
    for c in range(NCORES):
        in_maps.append(
            {
                "xt": xt_host,
                "w": np.ascontiguousarray(w_host[:, :, c * FC : (c + 1) * FC]),
                "bias": np.ascontiguousarray(
                    np.broadcast_to(b32[c * FC : (c + 1) * FC], (P, FC))
                ),
            }
        )
    return in_maps


def gather(results):
    out = np.empty((T, F), dtype=np.float32)
    for c in range(NCORES):
        y_c = results[c]["y"]  # [P, NTT, NFC, FD]
        out[:, c * FC : (c + 1) * FC] = (
            y_c.reshape(P, NTT, FC).transpose(1, 0, 2).reshape(T, FC)
        )
    return out.reshape(B, S, F)


def kernel(**inputs):
    from concourse import bass_utils

    nc = _get_compiled()
    in_maps = prep_inputs(inputs["inputs"], inputs["kernel"], inputs["bias"])
    last_err = None
    for _attempt in range(3):
        try:
            res = bass_utils.run_bass_kernel_spmd(
                nc, in_maps, core_ids=list(range(NCORES)), trace=False
            )
            return gather(res.results)
        except Exception as e:
            last_err = e
    raise last_err
